# revision 13
# baseline (speedup 1.0000x reference)
"""BottomRightPool (2D cummax) Trainium2 Bass kernel.

pool[b,c,i,j] = max(x[b,c,:i+1,:j+1])  ==  cummax over H, then over W.

Row recurrence (one op per row): out[i] = runmax_w(max(x[i], out[i-1])),
because cummax_w(out[i-1]) == out[i-1].  Done by a custom DVE op CUMMAX2
(Spec(body=scan(MAX, maxx(Src0, Src1)))): the scan combine is a single ALU
stage with same-stage feedback -> 1 elem/cycle, vs 2 elem/cycle for the stock
tensor_tensor_scan (whose op0->op1 state path costs a bubble per element).

Measured on-HW (loop-slope): CUMMAX2 236 ns/op at [128,128] with 4 interleaved
chains (decode-rate bound); stock scan 348 ns.  DMA in+out concurrently runs
~300 GB/s/core with 16 KiB per-partition segments.

Everything moves as bf16 (rel-err ~2^-9 from input rounding only -- max is
exact; bf16 keeps full f32 exponent range so tiny values stay accurate,
unlike fp16):  host converts f32->bf16, device reads/writes bf16 (halves HBM
traffic), host upcasts the result.

Layout (per core, data-parallel over the 4096 (b,c) slices):
  - 512 slices/core; 4 chunks x [128 partitions = slices, HB*128 free=(h,w)].
  - h-blocks of HB=32 rows; row ops interleave across the 4 chunks so
    adjacent DVE ops are independent (dependency distance 4); the per-chunk
    recurrence chains across h-blocks via in1 = prev block's last output row.
  - DMA pieces are split (IN_SPLIT=2 -> 16-row input pieces, OUT_SPLIT=4 ->
    8-row output pieces issued as soon as each row-group completes): compute
    starts after the first piece lands and outputs drain incrementally, which
    measured ~12% faster than whole-block transfers despite smaller segments.
"""

import numpy as np

N_CORES = 8
B, C, H, W = 16, 256, 128, 128
S = B * C                    # 4096 independent (b,c) slices
SPC = S // N_CORES           # 512 slices per core
CHUNK = 128                  # slices per tile (partition dim)
HB = 32                      # rows per h-block tile

_CACHE = {}


def _get_cummax2():
    """Register (once) the custom DVE op  out = runmax(max(in0, in1))."""
    if "op" in _CACHE:
        return _CACHE["op"]
    import concourse.dve_ops as dve_ops
    from concourse.dve_ops import DveOp
    from concourse.dve_spec import AluOp, Spec, Src0, Src1, lower, maxx, scan
    from concourse.dve_uop import DveOpSpec

    for op in dve_ops.OPS:
        if op.name == "CUMMAX2":
            _CACHE["op"] = op
            return op

    spec = Spec(
        body=scan(AluOp.MAX, maxx(Src0, Src1)),
        reference=lambda in0, in1, s0, s1, imm2: np.maximum.accumulate(
            np.maximum(in0.astype(np.float32), in1.astype(np.float32)), axis=-1
        ),
    )
    row = dve_ops._CUSTOM_DVE_ROW_BASE + len(dve_ops.OPS)
    shas = {}
    for ver in ("v3", "v4"):
        try:
            tmp = DveOpSpec(
                name="CUMMAX2", opcode=row, uops=lower(spec, ver=ver), rd1_en=True
            )
            shas[ver] = tmp.sha(ver)
        except Exception:
            pass
    assert shas, "lower() failed for every DveVer"
    op = DveOp("CUMMAX2", spec, subdim=False, uops_sha=shas)
    dve_ops.OPS.append(op)
    dve_ops._SUB_OPCODE_FOR_NAME[op.name] = row
    _CACHE["op"] = op
    return op


LAYOUT = "flat"              # "flat": dram [SPC,H,W]; "merged": [128, hb, c, HB, W]
BUFS_A = 3                   # input-tile pool depth
BUFS_B = 3                   # output-tile pool depth
OUT_SPLIT = 4                # out-DMA pieces per (chunk, block)
IN_SPLIT = 2                 # in-DMA pieces per (chunk, block)
MIX_RINGS = False            # alternate in/out pieces across both HWDGE rings
ONE_RING = False             # issue all DMAs on the sync ring


def _build_nc(repeat=None, mode="full"):
    if LAYOUT == "merged":
        return _build_nc_merged(repeat, mode)
    return _build_nc_flat(repeat, mode)


def _build_nc_merged(repeat=None, mode="full"):
    import concourse.mybir as mybir
    import concourse.tile as tile
    from concourse import bacc

    OP = _get_cummax2()
    nc = bacc.Bacc(None, target_bir_lowering=False)
    DT = mybir.dt.bfloat16
    n_chunks = SPC // CHUNK
    n_hb = H // HB
    xd = nc.dram_tensor("x", [CHUNK, n_hb, n_chunks, HB, W], DT, kind="ExternalInput")
    od = nc.dram_tensor("out", [CHUNK, n_hb, n_chunks, HB, W], DT, kind="ExternalOutput")

    with tile.TileContext(nc) as tc:
        with tc.tile_pool(name="ina", bufs=3) as pa, tc.tile_pool(
            name="outb", bufs=3
        ) as pb:

            def body():
                prev = [None] * n_chunks
                for hb in range(n_hb):
                    A = pa.tile([CHUNK, n_chunks * HB * W], DT, name="A")
                    Bt = pb.tile([CHUNK, n_chunks * HB * W], DT, name="B")
                    nc.sync.dma_start(
                        out=A[:],
                        in_=xd[:, hb].rearrange("p c h w -> p (c h w)"),
                    )
                    if mode != "dma":
                        for r in range(HB):
                            for c in range(n_chunks):
                                row = slice((c * HB + r) * W, (c * HB + r + 1) * W)
                                if r == 0 and prev[c] is None:
                                    in1 = A[:, row]
                                elif r == 0:
                                    in1 = prev[c]
                                else:
                                    in1 = Bt[:, (c * HB + r - 1) * W : (c * HB + r) * W]
                                nc.vector._custom_dve(
                                    OP, out=Bt[:, row], in0=A[:, row], in1=in1
                                )
                        for c in range(n_chunks):
                            prev[c] = Bt[:, (c * HB + HB - 1) * W : (c * HB + HB) * W]
                    if mode != "noout":
                        src = A if mode == "dma" else Bt
                        nc.scalar.dma_start(
                            out=od[:, hb].rearrange("p c h w -> p (c h w)"),
                            in_=src[:],
                        )

            if repeat is None:
                body()
            else:
                with tc.For_i(0, repeat, 1):
                    body()
    nc.compile()
    return nc


def _build_nc_flat(repeat=None, mode="full"):
    """Build the per-core Bass program. repeat=None emits the plain kernel;
    repeat=R wraps the whole workload in a hardware For_i loop (benchmarking
    only — output is just rewritten R times)."""
    import concourse.mybir as mybir
    import concourse.tile as tile
    from concourse import bacc

    OP = _get_cummax2()
    nc = bacc.Bacc(None, target_bir_lowering=False)
    DT = mybir.dt.bfloat16
    xd = nc.dram_tensor("x", [SPC, H, W], DT, kind="ExternalInput")
    od = nc.dram_tensor("out", [SPC, H, W], DT, kind="ExternalOutput")

    n_chunks = SPC // CHUNK   # 4 interleaved row-recurrence chains

    with tile.TileContext(nc) as tc:
        with tc.tile_pool(name="ina", bufs=BUFS_A) as pa, tc.tile_pool(
            name="outb", bufs=BUFS_B
        ) as pb:

            def body():
                prev = [None] * n_chunks  # pool row above current block
                for hb in range(H // HB):
                    h0 = hb * HB
                    tiles = []
                    for c in range(n_chunks):
                        s0 = c * CHUNK
                        A = pa.tile([CHUNK, HB * W], DT, name=f"A{c}")
                        Bt = pb.tile([CHUNK, HB * W], DT, name=f"B{c}")
                        HGI = HB // IN_SPLIT
                        for g in range(IN_SPLIT):
                            _e = (nc.sync, nc.scalar)[g % 2] if MIX_RINGS else nc.sync
                            _e.dma_start(
                                out=A[:, g * HGI * W : (g + 1) * HGI * W],
                                in_=xd[
                                    s0 : s0 + CHUNK,
                                    h0 + g * HGI : h0 + (g + 1) * HGI,
                                ].rearrange("s h w -> s (h w)"),
                            )
                        tiles.append((A, Bt))
                    HG = HB // OUT_SPLIT
                    if mode != "dma":
                        for r in range(HB):
                            row = slice(r * W, (r + 1) * W)
                            for c, (A, Bt) in enumerate(tiles):
                                if r == 0 and prev[c] is None:
                                    in1 = A[:, row]
                                elif r == 0:
                                    in1 = prev[c]
                                else:
                                    in1 = Bt[:, (r - 1) * W : r * W]
                                nc.vector._custom_dve(
                                    OP, out=Bt[:, row], in0=A[:, row], in1=in1
                                )
                            if mode != "noout" and (r + 1) % HG == 0:
                                g = r + 1 - HG
                                for c, (A, Bt) in enumerate(tiles):
                                    s0 = c * CHUNK
                                    _e = (
                                        (nc.scalar, nc.sync)[((r + 1) // HG + c) % 2]
                                        if MIX_RINGS
                                        else (nc.sync if ONE_RING else nc.scalar)
                                    )
                                    _e.dma_start(
                                        out=od[
                                            s0 : s0 + CHUNK, h0 + g : h0 + r + 1
                                        ].rearrange("s h w -> s (h w)"),
                                        in_=Bt[:, g * W : (r + 1) * W],
                                    )
                        for c, (A, Bt) in enumerate(tiles):
                            prev[c] = Bt[:, (HB - 1) * W : HB * W]
                    else:
                        for c, (A, Bt) in enumerate(tiles):
                            s0 = c * CHUNK
                            nc.scalar.dma_start(
                                out=od[s0 : s0 + CHUNK, h0 : h0 + HB].rearrange(
                                    "s h w -> s (h w)"
                                ),
                                in_=A[:],
                            )

            if repeat is None:
                body()
            else:
                with tc.For_i(0, repeat, 1):
                    body()
    nc.compile()
    return nc


def make_runner(nc):
    """Compile once; return run(in_maps) plus the raw jitted callable.

    Mirrors concourse.bass2jax.run_bass_via_pjrt's multi-core path but keeps
    the jitted executable so repeated calls don't re-trace/re-compile.
    """
    import jax
    import concourse.mybir as mybir
    from jax.sharding import Mesh, PartitionSpec
    from jax.experimental.shard_map import shard_map
    from concourse.bass2jax import (
        _bass_exec_p,
        install_neuronx_cc_hook,
        partition_id_tensor,
    )

    install_neuronx_cc_hook()
    assert nc.dbg_addr is None
    partition_name = nc.partition_id_tensor.name if nc.partition_id_tensor else None

    in_names, out_names, out_avals, zero_outs = [], [], [], []
    for alloc in nc.m.functions[0].allocations:
        if not isinstance(alloc, mybir.MemoryLocationSet):
            continue
        name = alloc.memorylocations[0].name
        if alloc.kind == "ExternalInput":
            if name == partition_name:
                continue
            in_names.append(name)
        elif alloc.kind == "ExternalOutput":
            out_names.append(name)
            shape = tuple(alloc.tensor_shape)
            dtype = mybir.dt.np(alloc.dtype)
            out_avals.append(jax.core.ShapedArray(shape, dtype))
            zero_outs.append(np.zeros(shape, dtype))
    n_params = len(in_names)
    n_outs = len(out_avals)
    all_in_names = in_names + out_names
    if partition_name is not None:
        all_in_names = all_in_names + [partition_name]
    donate = tuple(range(n_params, n_params + n_outs))

    def _body(*args):
        operands = list(args)
        if partition_name is not None:
            operands.append(partition_id_tensor())
        outs = _bass_exec_p.bind(
            *operands,
            out_avals=tuple(out_avals),
            in_names=tuple(all_in_names),
            out_names=tuple(out_names),
            lowering_input_output_aliases=(),
            sim_require_finite=True,
            sim_require_nnan=True,
            nc=nc,
        )
        return tuple(outs)

    devices = jax.devices()[:N_CORES]
    mesh = Mesh(np.asarray(devices), ("core",))
    sharded = jax.jit(
        shard_map(
            _body,
            mesh=mesh,
            in_specs=(PartitionSpec("core"),) * (n_params + n_outs),
            out_specs=(PartitionSpec("core"),) * n_outs,
            check_rep=False,
        ),
        donate_argnums=donate,
        keep_unused=True,
    )

    def make_args(in_maps):
        concat_in = [
            np.concatenate([np.asarray(m[name]) for m in in_maps], axis=0)
            for name in in_names
        ]
        concat_zeros = [
            np.zeros((N_CORES * z.shape[0], *z.shape[1:]), z.dtype)
            for z in zero_outs
        ]
        return concat_in + concat_zeros

    def run(in_maps):
        out_arrs = sharded(*make_args(in_maps))
        return [
            {
                name: np.asarray(out_arrs[i]).reshape(
                    N_CORES, *out_avals[i].shape
                )[c]
                for i, name in enumerate(out_names)
            }
            for c in range(N_CORES)
        ]

    return run, sharded, make_args


def _to_bf16(x: np.ndarray) -> np.ndarray:
    import ml_dtypes

    return np.ascontiguousarray(x, dtype=np.float32).astype(ml_dtypes.bfloat16)


def _pack(xcore: np.ndarray) -> np.ndarray:
    """[SPC, H, W] -> [128, n_hb, n_chunks, HB, W] (merged layout)."""
    nch, nhb = SPC // CHUNK, H // HB
    return np.ascontiguousarray(
        xcore.reshape(nch, CHUNK, nhb, HB, W).transpose(1, 2, 0, 3, 4)
    )


def _unpack(ocore: np.ndarray) -> np.ndarray:
    """[128, n_hb, n_chunks, HB, W] -> [SPC, H, W]."""
    nch, nhb = SPC // CHUNK, H // HB
    return ocore.transpose(2, 0, 1, 3, 4).reshape(SPC, H, W)


def _run(x: np.ndarray, trace: bool = False):
    """Returns (full_output, exec_time_ns_or_None)."""
    if "runner" not in _CACHE:
        nc = _build_nc()
        _CACHE["runner"] = make_runner(nc)[0]
    run = _CACHE["runner"]
    xf = _to_bf16(x).reshape(S, H, W)
    if LAYOUT == "merged":
        in_maps = [
            {"x": _pack(xf[k * SPC : (k + 1) * SPC])} for k in range(N_CORES)
        ]
        results = run(in_maps)
        out = np.concatenate(
            [_unpack(r["out"]) for r in results], axis=0
        )
    else:
        in_maps = [{"x": xf[k * SPC : (k + 1) * SPC]} for k in range(N_CORES)]
        results = run(in_maps)
        out = np.concatenate([r["out"] for r in results], axis=0)
    return out.astype(np.float32).reshape(B, C, H, W), None


def kernel(x: np.ndarray) -> np.ndarray:
    return _run(x)[0]
